# revision 23
# baseline (speedup 1.0000x reference)
"""BigramAttn Trainium2 kernel (8-core SPMD, raw Bass).

Reference computation (per batch b):
  e[0]   = sum_k enc[0,k] * h[k]
  e[s]   = sum_k (enc[s-1,:] @ M)[k] * h[k] * enc[s,k]          (s >= 1)
  e[s]  += sum_{k<3} (h @ affect)[k] * emb[s,k]
  out    = softmax(e)                                            # over s
Sharding: data-parallel over batch B=32 across 8 cores (4 batches/core).

Per core, steps ordered (chunk c, batch b) so that 4 consecutive steps
cover all batches of one s-chunk:
  A_T[k, t]  = sum_j M[j,k] * encT_b[j, s0+t-1]      (PE fp32r, 16 MMs/step)
  P[k, t]    = (A_T[k, t] * h_b[k]) * encT_b[k, s0+t]   (DVE stt, x4)
  Q[p, 2t]   = P_0+P_1, P_2+P_3                      (GpSimd, 2 tensor_add;
               Pool rejects scalar-ptr ops, so h folds in the DVE stt;
               real-HW Pool adds cost ~1.3us so only 2 fit per step)
  ps_e[b, t] = ind_b^T @ Q_01 + ind_b^T @ Q_23       (PE, 2 accum MMs/step;
               ind_b [128,4] is 1 in col b so the 4 steps of chunk c fill
               all 4 batch rows of ONE psum bank)
  e4[:, c]   = ps_e + e_aff[:, c]                    (DVE, 1 add per chunk)
  softmax over 4096 logits per batch, batched as [4, 4096]; per-chunk
  running maxes hide the max reduce; the Exp table preloads at startup.

M is shared/resident in SBUF (1 MB once, in 4 [128,512] tensors -- LDWEIGHTS
from a wide tensor measured 225 ns vs 167, unhiding it behind the 213 ns MM
stream); the affect energy e_aff[b,s] is precomputed on host (tiny: B*S*3
MACs). Engine compute APs may only start at partitions 0/32/64/96, hence the
chunk-major ordering that lands e rows contiguously at partitions 0..3.
Setup DMAs issue from three engines in parallel (ACT: M, DVE: small tensors,
SP: enc chunks) -- each HWDGE issue costs ~0.7 us, serial on one queue.

Host pre-transposes the enc shard to [4, 512, 4096]: the PE contracts over
partitions so H must land on partitions; DMA-transpose is 2-byte-only on
trn2. All matmuls run float32r (full PE rate at N>=256; bf16 inputs measured
3.7e-2 rel err vs the 2e-2 gate -- logits are +-100, so fp32r is required).
fp32r ISA rules: even moving-column counts, 8B-aligned dst at partition 0,
fp32r-tagged producers end to end.

This walrus build accepts exactly ONE semaphore wait per instruction, so the
kernel is raw Bass: per-engine programs, counting semaphores, standalone
waits. Engines pipeline, so same-engine read-after-write needs explicit
self-sync semaphores. DMA completions may reorder across transfers, so chunk
DMAs chain on per-lane semaphores.
"""

import functools

import numpy as np

import concourse.bass as bass
from concourse import mybir
from concourse.bass_utils import run_bass_kernel_spmd

S, B, H = 4096, 32, 512
NCORES = 8
BC = B // NCORES          # batches per core = 4
NK = H // 128             # h-chunks = 4
CH = 512                  # s-chunk width
CW = CH + 1               # chunk tile block width (1-col halo)
NCH = S // CH             # s-chunks per batch = 8
NBC = BC * NCH            # chunk-steps per core = 32
NSLOT = 4                 # enc chunk tiles; slot == batch with (c,b) order
NLANE = 4                 # DMA completion-ordering lanes; lane == batch
NSET = 3                  # small setup DMAs on dma_set (ACT-issued)

F32 = mybir.dt.float32
F32R = mybir.dt.float32r


@functools.lru_cache(maxsize=1)
def _build():
    nc = bass.Bass("TRN2", target_bir_lowering=False, debug=False)

    enc_t = nc.dram_tensor("enc_t", [BC, H, S], F32R, kind="ExternalInput").ap()
    m_d = nc.dram_tensor("m_d", [NK, 128, H], F32R, kind="ExternalInput").ap()
    ht_d = nc.dram_tensor("ht_d", [128, BC * NK], F32,
                          kind="ExternalInput").ap()
    ind_d = nc.dram_tensor("ind_d", [128, BC * BC], F32R,
                           kind="ExternalInput").ap()
    one_v = nc.dram_tensor("one_v", [128, NSLOT * NK], F32R,
                           kind="ExternalInput").ap()
    eaff_d = nc.dram_tensor("eaff_d", [BC, S], F32, kind="ExternalInput").ap()
    out = nc.dram_tensor("out", [BC, S], F32, kind="ExternalOutput").ap()

    # SBUF (~110 KB/partition of ~192 usable)
    enc_all = nc.alloc_sbuf_tensor("enc", [128, NSLOT * NK * CW], F32R).ap()

    def enc_v(sl):
        return enc_all[:, sl * NK * CW:(sl + 1) * NK * CW]

    m_sb = [nc.alloc_sbuf_tensor(f"m{j}", [128, H], F32R).ap()
            for j in range(NK)]
    ht_sb = nc.alloc_sbuf_tensor("ht", [128, BC * NK], F32).ap()
    ind_sb = nc.alloc_sbuf_tensor("ind", [128, BC * BC], F32R).ap()
    ones_sb = nc.alloc_sbuf_tensor("ones", [128, NSLOT * NK], F32R).ap()
    eaff_sb = nc.alloc_sbuf_tensor("eaff", [BC, S], F32).ap()
    p_sb = [nc.alloc_sbuf_tensor(f"p{i}", [128, NK * CH], F32R).ap()
            for i in range(2)]
    q_sb = [nc.alloc_sbuf_tensor(f"q{i}", [128, 2 * CH], F32R).ap()
            for i in range(2)]
    e4_sb = nc.alloc_sbuf_tensor("e4", [BC, S], F32).ap()
    ex4_sb = nc.alloc_sbuf_tensor("ex4", [BC, S], F32).ap()
    o4_sb = nc.alloc_sbuf_tensor("o4", [BC, S], F32).ap()
    pm_sb = nc.alloc_sbuf_tensor("pm", [BC, NCH], F32).ap()
    nmx_sb = nc.alloc_sbuf_tensor("nmx", [BC, 1], F32).ap()
    sm_sb = nc.alloc_sbuf_tensor("sm", [BC, 1], F32).ap()
    rs_sb = nc.alloc_sbuf_tensor("rs", [BC, 1], F32).ap()
    scr_sb = nc.alloc_sbuf_tensor("scr", [BC, 4], F32).ap()

    # PSUM: A region 4 banks + 2 e banks = 6 of 8
    ps_a = nc.alloc_psum_tensor("psA", [128, NK * CH], F32).ap()
    ps_e = [nc.alloc_psum_tensor(f"psE{i}", [BC, CH], F32).ap()
            for i in range(2)]

    dma_m = nc.alloc_semaphore("dma_m")      # M matrix DMAs (4, ACT-issued)
    dma_set = nc.alloc_semaphore("dma_set")  # small setup DMAs (ACT-issued)
    dma_sd = nc.alloc_semaphore("dma_sd")    # slot col-0 seed DMA
    dma_ef = nc.alloc_semaphore("dma_ef")    # e_aff DMA (needed ~step 5)
    dma_ln = [nc.alloc_semaphore(f"dma_ln{k}") for k in range(NLANE)]
    dma_out = nc.alloc_semaphore("dma_out")
    pe_mm = nc.alloc_semaphore("pe_mm")      # +1 per kt MM-group (4/step)
    pe_red = nc.alloc_semaphore("pe_red")    # +1 per step reduce MM
    dve_pm = nc.alloc_semaphore("dve_pm")    # +1 per P-mul pair (2/step)
    dve_ms = nc.alloc_semaphore("dve_ms")    # +1 per DVE col-0 psum seed
    gp_q = nc.alloc_semaphore("gp_q")        # +2 per step (independent folds)
    dve_e = nc.alloc_semaphore("dve_e")      # +1 per chunk e add (psum->e4)
    dve_px = nc.alloc_semaphore("dve_px")    # +1 per chunk pm max
    dve_sm = nc.alloc_semaphore("dve_sm")    # +1 reciprocal done (self-sync)
    dve_fin = nc.alloc_semaphore("dve_fin")  # +1 nmx, +1 per scale half
    act_ex = nc.alloc_semaphore("act_ex")    # +1 exp done

    with nc.Block() as blk:
        # --- SP: enc chunk + output DMAs ---
        @blk.sync
        def _(sync):
            for b in range(BC):
                # round 0 is paced so PE's critical bytes land first: chunk
                # (0,0) streams alongside M; (0,1) holds until M is in; each
                # later chunk holds for its predecessor (no bandwidth split)
                if b == 1:
                    sync.wait_ge(dma_m, 16 * NK)
                elif b >= 2:
                    sync.wait_ge(dma_ln[b - 1], 16)
                dst3 = enc_v(b).rearrange(
                    "p (k w) -> p k w", k=NK)[:, :, 1:CW]
                src3 = enc_t[b, :, 0:CH].rearrange("(k p) s -> p k s", p=128)
                sync.dma_start(dst3, src3).then_inc(dma_ln[b], 16)
            # chunk DMAs: ONE per step (c,b); slot/lane == batch b
            for c in range(1, NCH):
                for b in range(BC):
                    st = c * BC + b
                    # slot reuse: consumers of chunk (c-1, b) done
                    sync.wait_ge(pe_mm, 4 * (st - NSLOT) + 4)
                    sync.wait_ge(dve_pm, 4 * (st - NSLOT) + 4)
                    # lane chain => ordered completions within the lane
                    sync.wait_ge(dma_ln[b], 16 * (st // NLANE))
                    # block kt col u holds s = c*CH - 1 + u
                    s0 = c * CH - 1
                    dst3 = enc_v(b).rearrange(
                        "p (k w) -> p k w", k=NK)[:, :, 0:CW]
                    src3 = enc_t[b, :, s0:s0 + CW].rearrange(
                        "(k p) s -> p k s", p=128)
                    sync.dma_start(dst3, src3).then_inc(dma_ln[b], 16)
            # output, quarter-pipelined with the final scale
            for qr in range(4):
                sync.wait_ge(dve_fin, 2 + qr)
                sync.dma_start(out[:, qr * S // 4:(qr + 1) * S // 4],
                               o4_sb[:, qr * S // 4:(qr + 1) * S // 4]) \
                    .then_inc(dma_out, 16)
            sync.wait_ge(dma_out, 64)

        # --- PE ---
        @blk.tensor
        def _(tensor):
            def pe_reduce(j):
                # step j = (c_, b_): ps_e[c_%2][b_, t] += ind_b^T @ Q
                # (ind_b col b_ = 1, other cols 0 -> rows b'!=b_ untouched;
                # the 4 steps of chunk c_ fill all 4 rows of one bank)
                c_, b_ = j // BC, j % BC
                if j == 0:
                    tensor.wait_ge(dma_set, 16 * NSET)  # ind in
                tensor.wait_ge(gp_q, 2 * j + 2)
                if b_ == 0 and c_ >= 2:
                    tensor.wait_ge(dve_e, c_ - 1)  # WAR on ps_e[c_%2]
                for hf in range(2):
                    mm_r = nc.tensor.matmul(
                        ps_e[c_ % 2][0:BC, 0:CH],
                        ind_sb[:, b_ * BC:(b_ + 1) * BC],
                        q_sb[j % 2][:, hf * CH:(hf + 1) * CH],
                        start=(b_ == 0 and hf == 0),
                        stop=(b_ == BC - 1 and hf == 1))
                mm_r.then_inc(pe_red, 1)

            tensor.wait_ge(dma_m, 16 * NK)
            tensor.wait_ge(dma_set, 16 * NSET)  # ht/ind/ones/seeds/eaff in
            for c in range(NCH):
                for b in range(BC):
                    st = c * BC + b
                    tensor.wait_ge(dma_ln[b],
                                   16 * (st // NLANE + 1))  # chunk tile in
                    for kt in range(NK):
                        g = 4 * st + kt
                        if g >= 4:  # WAR on psA bank kt: P-stt done
                            tensor.wait_ge(dve_pm, g - 4 + 1)
                        for j in range(NK):
                            mm = nc.tensor.matmul(
                                ps_a[:, kt * CH:(kt + 1) * CH],
                                m_sb[j][:, kt * 128:(kt + 1) * 128],
                                enc_v(b)[:, j * CW:j * CW + CH],
                                start=(j == 0), stop=(j == NK - 1),
                            )
                        mm.then_inc(pe_mm, 1)
                    # deferred reduce of the PREVIOUS step: its fold chain
                    # finished during this step's MM groups -> no PE stall
                    if st >= 1:
                        pe_reduce(st - 1)
            pe_reduce(NBC - 1)

        # --- DVE ---
        @blk.vector
        def _(vector):
            def e_add(c_):
                # e4[:, chunk c_] = ps_e[c_%2] + e_aff[:, chunk c_]
                if c_ == 0:
                    vector.wait_ge(dma_ef, 16)
                vector.wait_ge(pe_red, BC * (c_ + 1))
                nc.vector.tensor_add(
                    e4_sb[0:BC, c_ * CH:(c_ + 1) * CH],
                    ps_e[c_ % 2][0:BC, 0:CH],
                    eaff_sb[0:BC, c_ * CH:(c_ + 1) * CH]) \
                    .then_inc(dve_e, 1)

            def e_max(c_):
                # running per-chunk max -> pm col; hides the softmax max.
                # DVE pipelines, so self-sync the e4 write before reading it
                vector.wait_ge(dve_e, c_ + 1)
                nc.vector.tensor_reduce(
                    pm_sb[:, c_:c_ + 1], e4_sb[0:BC, c_ * CH:(c_ + 1) * CH],
                    mybir.AxisListType.X, mybir.AluOpType.max) \
                    .then_inc(dve_px, 1)

            vector.wait_ge(dma_set, 16 * NSET)
            n_ms = 0
            for c in range(NCH):
                for b in range(BC):
                    st = c * BC + b
                    if st >= 2:  # WAR on p[st%2]: folds of st-2 done
                        vector.wait_ge(gp_q, 2 * st - 2)
                    for kt in range(NK):
                        vector.wait_ge(pe_mm, 4 * st + kt + 1)
                        if c == 0:
                            # psum col 0 of the bank := 1 (A'[-1] = ones)
                            nc.vector.tensor_copy(
                                ps_a[:, kt * CH:kt * CH + 1],
                                ones_sb[:, kt:kt + 1]) \
                                .then_inc(dve_ms, 1)
                            n_ms += 1
                            vector.wait_ge(dve_ms, n_ms)
                        # P_kt = (A_kt * h_b_kt) * enc_kt  (h folded here)
                        nc.vector.scalar_tensor_tensor(
                            p_sb[st % 2][:, kt * CH:(kt + 1) * CH],
                            ps_a[:, kt * CH:(kt + 1) * CH],
                            ht_sb[:, b * NK + kt:b * NK + kt + 1],
                            enc_v(b)[:, kt * CW + 1:kt * CW + CW],
                            mybir.AluOpType.mult, mybir.AluOpType.mult) \
                            .then_inc(dve_pm, 1)
                    # previous chunk's e rows: reduce(4c-1) retires during
                    # step (c,0) on PE; split the add and the max across two
                    # steps so neither delays this step's P-stt chain
                    if b == 1 and c >= 1:
                        e_add(c - 1)
                    if b == 2 and c >= 1:
                        e_max(c - 1)
            # chunk 7's pm comes straight from psum, off the e_add chain
            # (the max may miss e_aff; any shift keeps softmax exact --
            # values stay far from fp32 overflow)
            c_l = NCH - 1
            vector.wait_ge(pe_red, BC * NCH)
            nc.vector.tensor_reduce(
                pm_sb[:, c_l:c_l + 1], ps_e[c_l % 2][0:BC, 0:CH],
                mybir.AxisListType.X, mybir.AluOpType.max) \
                .then_inc(dve_px, 1)
            nc.vector.tensor_add(
                e4_sb[0:BC, c_l * CH:(c_l + 1) * CH],
                ps_e[c_l % 2][0:BC, 0:CH],
                eaff_sb[0:BC, c_l * CH:(c_l + 1) * CH]) \
                .then_inc(dve_e, 1)
            # softmax tail (batched on [4, S]); maxes already folded in pm
            vector.wait_ge(dve_px, NCH)  # self-sync pm writes
            nc.vector.tensor_reduce(nmx_sb[:], pm_sb[:], mybir.AxisListType.X,
                                    mybir.AluOpType.max, negate=True) \
                .then_inc(dve_fin, 1)   # "nmx ready" (ACT waits 1)
            vector.wait_ge(act_ex, 1)
            nc.vector.reciprocal(rs_sb[:], sm_sb[:]).then_inc(dve_sm, 1)
            vector.wait_ge(dve_sm, 1)
            for qr in range(4):  # quarter-pipelined with the out DMAs
                nc.vector.tensor_scalar_mul(
                    o4_sb[:, qr * S // 4:(qr + 1) * S // 4],
                    ex4_sb[:, qr * S // 4:(qr + 1) * S // 4],
                    rs_sb[0:BC, 0:1]) \
                    .then_inc(dve_fin, 1)   # dve_fin==2+qr -> SP DMA qr

        # --- GpSimd: h-weighted fold of P blocks 2,1,0 into Q ---
        @blk.gpsimd
        def _(gpsimd):
            for c in range(NCH):
                for b in range(BC):
                    st = c * BC + b
                    qbuf = q_sb[st % 2]
                    pbuf = p_sb[st % 2]
                    # WAR: reduce(st-2) finished reading q
                    if st >= 2:
                        gpsimd.wait_ge(pe_red, st - 1)
                    for hf in range(2):  # independent folds, no RMW chain
                        gpsimd.wait_ge(dve_pm, 4 * st + 2 * hf + 2)
                        nc.gpsimd.tensor_add(
                            qbuf[:, hf * CH:(hf + 1) * CH],
                            pbuf[:, 2 * hf * CH:(2 * hf + 1) * CH],
                            pbuf[:, (2 * hf + 1) * CH:(2 * hf + 2) * CH]) \
                            .then_inc(gp_q, 1)

        # --- ACT: M DMAs, exp table preload, then exp ---
        @blk.scalar
        def _(scalar):
            # queue order = PE's need order: seed (step-0 MMs read enc col
            # 0), M, then the smalls nobody needs until step ~1
            with nc.allow_non_contiguous_dma(
                    reason="tiny one-time slot col-0 seeds (16 elems)"):
                nc.scalar.dma_start(
                    enc_all.rearrange("p (s k w) -> p s k w",
                                      s=NSLOT, k=NK)[:, :, :, 0:1],
                    one_v.rearrange("p (s k o) -> p s k o", s=NSLOT, o=1)) \
                    .then_inc(dma_sd, 16)
            nc.scalar.dma_start(ht_sb[:], ht_d[:]).then_inc(dma_set, 16)
            nc.scalar.dma_start(ind_sb[:], ind_d[:]).then_inc(dma_set, 16)
            nc.scalar.dma_start(ones_sb[:], one_v[:]).then_inc(dma_set, 16)
            for j in range(NK):
                nc.scalar.dma_start(m_sb[j][:], m_d[j]).then_inc(dma_m, 16)
            nc.scalar.dma_start(eaff_sb[:], eaff_d[:]).then_inc(dma_ef, 16)
            scalar.wait_ge(dma_m, 16 * NK)
            nc.scalar.activation(scr_sb[:, 0:2], m_sb[0][0:BC, 0:2],
                                 mybir.ActivationFunctionType.Exp)
            scalar.wait_ge(dve_fin, 1)  # nmx ready (also: all e4 written)
            nc.scalar.activation(ex4_sb[:], e4_sb[:],
                                 mybir.ActivationFunctionType.Exp,
                                 bias=nmx_sb[0:BC, 0:1],
                                 accum_out=sm_sb[0:BC, 0:1]) \
                .then_inc(act_ex, 1)

    # no end-of-program sem clears: each PJRT execution starts with fresh
    # semaphore state (verified: 3 back-to-back executions of one loaded NEFF
    # each gave correct, input-scaled results).
    return nc


def _shard_host(hidden, encoder_outputs, embedding, bigram_matrix, affect_matrix):
    """Build per-core input maps. Only layout/scaling prep happens here."""
    h = np.asarray(hidden, dtype=np.float32)[0]              # [B, H]
    enc = np.asarray(encoder_outputs, dtype=np.float32)      # [S, B, H]
    emb = np.asarray(embedding, dtype=np.float32)            # [S, B, 3]
    m = np.asarray(bigram_matrix, dtype=np.float32)
    aff = np.asarray(affect_matrix, dtype=np.float32)        # [H, 3]

    enc_bhs = np.ascontiguousarray(enc.transpose(1, 2, 0))   # [B, H, S]
    # m_d[j, p, k] = M[j*128 + p, k]  (j-chunk on partitions)
    m_d = np.ascontiguousarray(m.reshape(NK, 128, H))
    # affect energy fully on host: e_aff[b, s] = (h_b @ aff) . emb[s, b]
    e_aff = np.einsum('bk,sbk->bs', h @ aff, emb)            # [B, S]
    # ind[p, b*BC + col] = 1 iff col == b (reduce stationary indicator)
    ind = np.zeros((128, BC * BC), dtype=np.float32)
    for b in range(BC):
        ind[:, b * BC + b] = 1.0

    in_maps = []
    for co in range(NCORES):
        b0 = co * BC
        # ht[p, b*NK + kt] = h[b0+b, kt*128 + p]
        ht = np.ascontiguousarray(
            h[b0:b0 + BC].reshape(BC, NK, 128).transpose(2, 0, 1)
            .reshape(128, BC * NK))
        in_maps.append({
            "enc_t": enc_bhs[b0:b0 + BC],                     # [BC, H, S]
            "m_d": m_d,
            "ht_d": ht,
            "ind_d": ind,
            "one_v": np.ones((128, NSLOT * NK), dtype=np.float32),
            "eaff_d": np.ascontiguousarray(e_aff[b0:b0 + BC]),
        })
    return in_maps


def kernel(hidden, encoder_outputs, embedding, bigram_matrix, affect_matrix,
           _want_results=False, _spmd_kwargs=None):
    nc = _build()
    in_maps = _shard_host(hidden, encoder_outputs, embedding,
                          bigram_matrix, affect_matrix)
    res = run_bass_kernel_spmd(nc, in_maps, core_ids=list(range(NCORES)),
                               **(_spmd_kwargs or {}))
    outp = np.empty((B, 1, S), dtype=np.float32)
    for co in range(NCORES):
        outp[co * BC:(co + 1) * BC, 0, :] = res.results[co]["out"]
    if _want_results:
        return outp, res
    return outp


# revision 28
# speedup vs baseline: 1.2158x; 1.2158x over previous
"""BigramAttn Trainium2 kernel (8-core SPMD, raw Bass).

Reference computation (per batch b):
  e[0]   = sum_k enc[0,k] * h[k]
  e[s]   = sum_k (enc[s-1,:] @ M)[k] * h[k] * enc[s,k]          (s >= 1)
  e[s]  += sum_{k<3} (h @ affect)[k] * emb[s,k]
  out    = softmax(e)                                            # over s
Sharding: data-parallel over batch B=32 across 8 cores (4 batches/core).

Per core, steps ordered (chunk c, batch b) so that 4 consecutive steps
cover all batches of one s-chunk:
  A_T[k, t]  = sum_j M[j,k] * encT_b[j, s0+t-1]      (PE fp32r, 16 MMs/step)
  P[k, t]    = (A_T[k, t] * h_b[k]) * encT_b[k, s0+t]   (DVE stt, x4)
  Q[p, 2t]   = P_0+P_1, P_2+P_3                      (GpSimd, 2 tensor_add;
               Pool rejects scalar-ptr ops, so h folds in the DVE stt;
               real-HW Pool adds cost ~1.3us so only 2 fit per step)
  ps_e[b, t] = ind_b^T @ Q_01 + ind_b^T @ Q_23       (PE, 2 accum MMs/step;
               ind_b [128,4] is 1 in col b so the 4 steps of chunk c fill
               all 4 batch rows of ONE psum bank)
  e4[:, c]   = ps_e + e_aff[:, c]                    (DVE, 1 add per chunk)
  softmax over 4096 logits per batch, batched as [4, 4096]; per-chunk
  running maxes hide the max reduce; the Exp table preloads at startup.

M is shared/resident in SBUF (1 MB once, in 4 [128,512] tensors -- LDWEIGHTS
from a wide tensor measured 225 ns vs 167, unhiding it behind the 213 ns MM
stream); the affect energy e_aff[b,s] is precomputed on host (tiny: B*S*3
MACs). Engine compute APs may only start at partitions 0/32/64/96, hence the
chunk-major ordering that lands e rows contiguously at partitions 0..3.
Setup DMAs issue from three engines in parallel (ACT: M, DVE: small tensors,
SP: enc chunks) -- each HWDGE issue costs ~0.7 us, serial on one queue.

Host pre-transposes the enc shard to [4, 512, 4096]: the PE contracts over
partitions so H must land on partitions; DMA-transpose is 2-byte-only on
trn2. All matmuls run float32r (full PE rate at N>=256; bf16 inputs measured
3.7e-2 rel err vs the 2e-2 gate -- logits are +-100, so fp32r is required).
fp32r ISA rules: even moving-column counts, 8B-aligned dst at partition 0,
fp32r-tagged producers end to end.

This walrus build accepts exactly ONE semaphore wait per instruction, so the
kernel is raw Bass: per-engine programs, counting semaphores, standalone
waits. Engines pipeline, so same-engine read-after-write needs explicit
self-sync semaphores. DMA completions may reorder across transfers, so chunk
DMAs chain on per-lane semaphores.
"""

import functools

import numpy as np

import concourse.bass as bass
from concourse import mybir
from concourse.bass_utils import run_bass_kernel_spmd

S, B, H = 4096, 32, 512
NCORES = 8
BC = B // NCORES          # batches per core = 4
NK = H // 128             # h-chunks = 4
CH = 512                  # s-chunk width
CW = CH + 1               # chunk tile block width (1-col halo)
NCH = S // CH             # s-chunks per batch = 8
NBC = BC * NCH            # chunk-steps per core = 32
NSLOT = 4                 # enc chunk tiles; slot == batch with (c,b) order
NLANE = 4                 # DMA completion-ordering lanes; lane == batch
NSET = 3                  # small setup DMAs on dma_set (ACT-issued)

F32 = mybir.dt.float32
F32R = mybir.dt.float32r


@functools.lru_cache(maxsize=1)
def _build():
    nc = bass.Bass("TRN2", target_bir_lowering=False, debug=False)

    enc_t = nc.dram_tensor("enc_t", [BC, H, S], F32R, kind="ExternalInput").ap()
    m_d = nc.dram_tensor("m_d", [NK, 128, H], F32R, kind="ExternalInput").ap()
    ht_d = nc.dram_tensor("ht_d", [128, BC * NK], F32,
                          kind="ExternalInput").ap()
    ind_d = nc.dram_tensor("ind_d", [128, BC * BC], F32R,
                           kind="ExternalInput").ap()
    one_v = nc.dram_tensor("one_v", [128, NSLOT * NK], F32R,
                           kind="ExternalInput").ap()
    eaff_d = nc.dram_tensor("eaff_d", [BC, S], F32, kind="ExternalInput").ap()
    out = nc.dram_tensor("out", [BC, S], F32, kind="ExternalOutput").ap()

    # SBUF (~110 KB/partition of ~192 usable)
    enc_all = nc.alloc_sbuf_tensor("enc", [128, NSLOT * NK * CW], F32R).ap()

    def enc_v(sl):
        return enc_all[:, sl * NK * CW:(sl + 1) * NK * CW]

    m_sb = [nc.alloc_sbuf_tensor(f"m{j}", [128, H], F32R).ap()
            for j in range(NK)]
    ht_sb = nc.alloc_sbuf_tensor("ht", [128, BC * NK], F32).ap()
    ind_sb = nc.alloc_sbuf_tensor("ind", [128, BC * BC], F32R).ap()
    ones_sb = nc.alloc_sbuf_tensor("ones", [128, NSLOT * NK], F32R).ap()
    eaff_sb = nc.alloc_sbuf_tensor("eaff", [BC, S], F32).ap()
    p_sb = [nc.alloc_sbuf_tensor(f"p{i}", [128, NK * CH], F32R).ap()
            for i in range(2)]
    q_sb = [nc.alloc_sbuf_tensor(f"q{i}", [128, 2 * CH], F32R).ap()
            for i in range(2)]
    e4_sb = nc.alloc_sbuf_tensor("e4", [BC, S], F32).ap()
    ex4_sb = nc.alloc_sbuf_tensor("ex4", [BC, S], F32).ap()
    o4_sb = nc.alloc_sbuf_tensor("o4", [BC, S], F32).ap()
    pm_sb = nc.alloc_sbuf_tensor("pm", [BC, NCH], F32).ap()
    nmx_sb = nc.alloc_sbuf_tensor("nmx", [BC, 1], F32).ap()
    sm_sb = nc.alloc_sbuf_tensor("sm", [BC, 1], F32).ap()
    rs_sb = nc.alloc_sbuf_tensor("rs", [BC, 1], F32).ap()
    scr_sb = nc.alloc_sbuf_tensor("scr", [BC, 4], F32).ap()

    # PSUM: A region 4 banks + 2 e banks = 6 of 8
    ps_a = nc.alloc_psum_tensor("psA", [128, NK * CH], F32).ap()
    ps_e = [nc.alloc_psum_tensor(f"psE{i}", [BC, CH], F32).ap()
            for i in range(2)]

    dma_m = nc.alloc_semaphore("dma_m")      # M matrix DMAs (4, ACT-issued)
    dma_set = nc.alloc_semaphore("dma_set")  # small setup DMAs (ACT-issued)
    dma_ef = nc.alloc_semaphore("dma_ef")    # e_aff DMA (needed ~step 5)
    dma_ln = [nc.alloc_semaphore(f"dma_ln{k}") for k in range(NLANE)]
    dma_out = nc.alloc_semaphore("dma_out")
    pe_mm = nc.alloc_semaphore("pe_mm")      # +1 per kt MM-group (4/step)
    pe_red = nc.alloc_semaphore("pe_red")    # +1 per step reduce MM
    dve_pm = nc.alloc_semaphore("dve_pm")    # +1 per P-mul pair (2/step)
    dve_ms = nc.alloc_semaphore("dve_ms")    # +1 per DVE col-0 psum seed
    gp_q = nc.alloc_semaphore("gp_q")        # +2 per step (independent folds)
    dve_e = nc.alloc_semaphore("dve_e")      # +1 per chunk e add (psum->e4)
    dve_px = nc.alloc_semaphore("dve_px")    # +1 per chunk pm max
    dve_sm = nc.alloc_semaphore("dve_sm")    # +1 reciprocal done (self-sync)
    dve_fin = nc.alloc_semaphore("dve_fin")  # +1 nmx, +1 per scale half
    act_ex = nc.alloc_semaphore("act_ex")    # +1 exp done

    with nc.Block() as blk:
        # --- SP: enc chunk + output DMAs ---
        @blk.sync
        def _(sync):
            # round-0 chunks back-to-back on SP's FIFO queue: in-order
            # completion at full per-transfer bandwidth, zero re-issue gaps
            # (completion-serializing them measured +2.4us dead time each)
            for b in range(BC):
                dst3 = enc_v(b).rearrange(
                    "p (k w) -> p k w", k=NK)[:, :, 1:CW]
                src3 = enc_t[b, :, 0:CH].rearrange("(k p) s -> p k s", p=128)
                sync.dma_start(dst3, src3).then_inc(dma_ln[b], 16)
            # chunk DMAs: ONE per step (c,b); slot/lane == batch b
            for c in range(1, NCH):
                for b in range(BC):
                    st = c * BC + b
                    # slot reuse: consumers of chunk (c-1, b) done
                    sync.wait_ge(pe_mm, 4 * (st - NSLOT) + 4)
                    sync.wait_ge(dve_pm, 4 * (st - NSLOT) + 4)
                    # lane chain => ordered completions within the lane
                    sync.wait_ge(dma_ln[b], 16 * (st // NLANE))
                    # block kt col u holds s = c*CH - 1 + u; round-0 tiles
                    # leave col 0 unwritten -- the MM reads garbage there and
                    # the psum column is replaced by the DVE ones-seed before
                    # any consumer (CoreSim pre-fills the tile in test only)
                    s0 = c * CH - 1
                    dst3 = enc_v(b).rearrange(
                        "p (k w) -> p k w", k=NK)[:, :, 0:CW]
                    src3 = enc_t[b, :, s0:s0 + CW].rearrange(
                        "(k p) s -> p k s", p=128)
                    sync.dma_start(dst3, src3).then_inc(dma_ln[b], 16)
            # output, quarter-pipelined with the final scale
            for qr in range(4):
                sync.wait_ge(dve_fin, 2 + qr)
                sync.dma_start(out[:, qr * S // 4:(qr + 1) * S // 4],
                               o4_sb[:, qr * S // 4:(qr + 1) * S // 4]) \
                    .then_inc(dma_out, 16)
            sync.wait_ge(dma_out, 64)

        # --- PE ---
        @blk.tensor
        def _(tensor):
            def pe_reduce(j):
                # step j = (c_, b_): ps_e[c_%2][b_, t] += ind_b^T @ Q
                # (ind_b col b_ = 1, other cols 0 -> rows b'!=b_ untouched;
                # the 4 steps of chunk c_ fill all 4 rows of one bank)
                c_, b_ = j // BC, j % BC
                if j == 0:
                    tensor.wait_ge(dma_set, 16 * NSET)  # ind in
                tensor.wait_ge(gp_q, 2 * j + 2)
                if b_ == 0 and c_ >= 2:
                    tensor.wait_ge(dve_e, c_ - 1)  # WAR on ps_e[c_%2]
                for hf in range(2):
                    mm_r = nc.tensor.matmul(
                        ps_e[c_ % 2][0:BC, 0:CH],
                        ind_sb[:, b_ * BC:(b_ + 1) * BC],
                        q_sb[j % 2][:, hf * CH:(hf + 1) * CH],
                        start=(b_ == 0 and hf == 0),
                        stop=(b_ == BC - 1 and hf == 1))
                mm_r.then_inc(pe_red, 1)

            tensor.wait_ge(dma_m, 16 * NK)
            for c in range(NCH):
                for b in range(BC):
                    st = c * BC + b
                    tensor.wait_ge(dma_ln[b],
                                   16 * (st // NLANE + 1))  # chunk tile in
                    for kt in range(NK):
                        g = 4 * st + kt
                        if g >= 4:  # WAR on psA bank kt: P-stt done
                            tensor.wait_ge(dve_pm, g - 4 + 1)
                        for j in range(NK):
                            mm = nc.tensor.matmul(
                                ps_a[:, kt * CH:(kt + 1) * CH],
                                m_sb[j][:, kt * 128:(kt + 1) * 128],
                                enc_v(b)[:, j * CW:j * CW + CH],
                                start=(j == 0), stop=(j == NK - 1),
                            )
                        mm.then_inc(pe_mm, 1)
                    # deferred reduce of the PREVIOUS step: its fold chain
                    # finished during this step's MM groups -> no PE stall
                    if st >= 1:
                        pe_reduce(st - 1)
            pe_reduce(NBC - 1)

        # --- DVE ---
        @blk.vector
        def _(vector):
            def e_add(c_):
                # e4[:, chunk c_] = ps_e[c_%2] + e_aff[:, chunk c_]
                if c_ == 0:
                    vector.wait_ge(dma_ef, 16)
                vector.wait_ge(pe_red, BC * (c_ + 1))
                nc.vector.tensor_add(
                    e4_sb[0:BC, c_ * CH:(c_ + 1) * CH],
                    ps_e[c_ % 2][0:BC, 0:CH],
                    eaff_sb[0:BC, c_ * CH:(c_ + 1) * CH]) \
                    .then_inc(dve_e, 1)

            def e_max(c_):
                # running per-chunk max -> pm col; hides the softmax max.
                # DVE pipelines, so self-sync the e4 write before reading it
                vector.wait_ge(dve_e, c_ + 1)
                nc.vector.tensor_reduce(
                    pm_sb[:, c_:c_ + 1], e4_sb[0:BC, c_ * CH:(c_ + 1) * CH],
                    mybir.AxisListType.X, mybir.AluOpType.max) \
                    .then_inc(dve_px, 1)

            vector.wait_ge(dma_set, 16 * NSET)
            n_ms = 0
            for c in range(NCH):
                for b in range(BC):
                    st = c * BC + b
                    if st >= 2:  # WAR on p[st%2]: folds of st-2 done
                        vector.wait_ge(gp_q, 2 * st - 2)
                    for kt in range(NK):
                        vector.wait_ge(pe_mm, 4 * st + kt + 1)
                        if c == 0:
                            # psum col 0 of the bank := 1 (A'[-1] = ones)
                            nc.vector.tensor_copy(
                                ps_a[:, kt * CH:kt * CH + 1],
                                ones_sb[:, kt:kt + 1]) \
                                .then_inc(dve_ms, 1)
                            n_ms += 1
                            vector.wait_ge(dve_ms, n_ms)
                        # P_kt = (A_kt * h_b_kt) * enc_kt  (h folded here)
                        nc.vector.scalar_tensor_tensor(
                            p_sb[st % 2][:, kt * CH:(kt + 1) * CH],
                            ps_a[:, kt * CH:(kt + 1) * CH],
                            ht_sb[:, b * NK + kt:b * NK + kt + 1],
                            enc_v(b)[:, kt * CW + 1:kt * CW + CW],
                            mybir.AluOpType.mult, mybir.AluOpType.mult) \
                            .then_inc(dve_pm, 1)
                    # previous chunk's e rows: reduce(4c-1) retires during
                    # step (c,0) on PE; split the add and the max across two
                    # steps so neither delays this step's P-stt chain
                    if b == 1 and c >= 1:
                        e_add(c - 1)
                    if b == 2 and c >= 1:
                        e_max(c - 1)
            # chunk 7's pm comes straight from psum, off the e_add chain
            # (the max may miss e_aff; any shift keeps softmax exact --
            # values stay far from fp32 overflow)
            c_l = NCH - 1
            vector.wait_ge(pe_red, BC * NCH)
            nc.vector.tensor_reduce(
                pm_sb[:, c_l:c_l + 1], ps_e[c_l % 2][0:BC, 0:CH],
                mybir.AxisListType.X, mybir.AluOpType.max) \
                .then_inc(dve_px, 1)
            nc.vector.tensor_add(
                e4_sb[0:BC, c_l * CH:(c_l + 1) * CH],
                ps_e[c_l % 2][0:BC, 0:CH],
                eaff_sb[0:BC, c_l * CH:(c_l + 1) * CH]) \
                .then_inc(dve_e, 1)
            # softmax tail (batched on [4, S]); maxes already folded in pm
            vector.wait_ge(dve_px, NCH)  # self-sync pm writes
            nc.vector.tensor_reduce(nmx_sb[:], pm_sb[:], mybir.AxisListType.X,
                                    mybir.AluOpType.max, negate=True) \
                .then_inc(dve_fin, 1)   # "nmx ready" (ACT waits 1)
            vector.wait_ge(act_ex, 1)
            nc.vector.reciprocal(rs_sb[:], sm_sb[:]).then_inc(dve_sm, 1)
            vector.wait_ge(dve_sm, 1)
            for qr in range(4):  # quarter-pipelined with the out DMAs
                nc.vector.tensor_scalar_mul(
                    o4_sb[:, qr * S // 4:(qr + 1) * S // 4],
                    ex4_sb[:, qr * S // 4:(qr + 1) * S // 4],
                    rs_sb[0:BC, 0:1]) \
                    .then_inc(dve_fin, 1)   # dve_fin==2+qr -> SP DMA qr

        # --- GpSimd: h-weighted fold of P blocks 2,1,0 into Q ---
        @blk.gpsimd
        def _(gpsimd):
            for c in range(NCH):
                for b in range(BC):
                    st = c * BC + b
                    qbuf = q_sb[st % 2]
                    pbuf = p_sb[st % 2]
                    # WAR: reduce(st-2) finished reading q
                    if st >= 2:
                        gpsimd.wait_ge(pe_red, st - 1)
                    for hf in range(2):  # independent folds, no RMW chain
                        gpsimd.wait_ge(dve_pm, 4 * st + 2 * hf + 2)
                        nc.gpsimd.tensor_add(
                            qbuf[:, hf * CH:(hf + 1) * CH],
                            pbuf[:, 2 * hf * CH:(2 * hf + 1) * CH],
                            pbuf[:, (2 * hf + 1) * CH:(2 * hf + 2) * CH]) \
                            .then_inc(gp_q, 1)

        # --- ACT: M DMAs, exp table preload, then exp ---
        @blk.scalar
        def _(scalar):
            # queue order = PE's need order: seed (step-0 MMs read enc col
            # 0), M, then the smalls nobody needs until step ~1
            for j in range(NK):
                nc.scalar.dma_start(m_sb[j][:], m_d[j]).then_inc(dma_m, 16)
            nc.scalar.dma_start(ht_sb[:], ht_d[:]).then_inc(dma_set, 16)
            nc.scalar.dma_start(ind_sb[:], ind_d[:]).then_inc(dma_set, 16)
            nc.scalar.dma_start(ones_sb[:], one_v[:]).then_inc(dma_set, 16)
            nc.scalar.dma_start(eaff_sb[:], eaff_d[:]).then_inc(dma_ef, 16)
            scalar.wait_ge(dma_m, 16 * NK)
            nc.scalar.activation(scr_sb[:, 0:2], m_sb[0][0:BC, 0:2],
                                 mybir.ActivationFunctionType.Exp)
            scalar.wait_ge(dve_fin, 1)  # nmx ready (also: all e4 written)
            nc.scalar.activation(ex4_sb[:], e4_sb[:],
                                 mybir.ActivationFunctionType.Exp,
                                 bias=nmx_sb[0:BC, 0:1],
                                 accum_out=sm_sb[0:BC, 0:1]) \
                .then_inc(act_ex, 1)

    # no end-of-program sem clears: each PJRT execution starts with fresh
    # semaphore state (verified: 3 back-to-back executions of one loaded NEFF
    # each gave correct, input-scaled results).
    return nc


def _shard_host(hidden, encoder_outputs, embedding, bigram_matrix, affect_matrix):
    """Build per-core input maps. Only layout/scaling prep happens here."""
    h = np.asarray(hidden, dtype=np.float32)[0]              # [B, H]
    enc = np.asarray(encoder_outputs, dtype=np.float32)      # [S, B, H]
    emb = np.asarray(embedding, dtype=np.float32)            # [S, B, 3]
    m = np.asarray(bigram_matrix, dtype=np.float32)
    aff = np.asarray(affect_matrix, dtype=np.float32)        # [H, 3]

    enc_bhs = np.ascontiguousarray(enc.transpose(1, 2, 0))   # [B, H, S]
    # m_d[j, p, k] = M[j*128 + p, k]  (j-chunk on partitions)
    m_d = np.ascontiguousarray(m.reshape(NK, 128, H))
    # affect energy fully on host: e_aff[b, s] = (h_b @ aff) . emb[s, b]
    e_aff = np.einsum('bk,sbk->bs', h @ aff, emb)            # [B, S]
    # ind[p, b*BC + col] = 1 iff col == b (reduce stationary indicator)
    ind = np.zeros((128, BC * BC), dtype=np.float32)
    for b in range(BC):
        ind[:, b * BC + b] = 1.0

    in_maps = []
    for co in range(NCORES):
        b0 = co * BC
        # ht[p, b*NK + kt] = h[b0+b, kt*128 + p]
        ht = np.ascontiguousarray(
            h[b0:b0 + BC].reshape(BC, NK, 128).transpose(2, 0, 1)
            .reshape(128, BC * NK))
        in_maps.append({
            "enc_t": enc_bhs[b0:b0 + BC],                     # [BC, H, S]
            "m_d": m_d,
            "ht_d": ht,
            "ind_d": ind,
            "one_v": np.ones((128, NSLOT * NK), dtype=np.float32),
            "eaff_d": np.ascontiguousarray(e_aff[b0:b0 + BC]),
        })
    return in_maps


def kernel(hidden, encoder_outputs, embedding, bigram_matrix, affect_matrix,
           _want_results=False, _spmd_kwargs=None):
    nc = _build()
    in_maps = _shard_host(hidden, encoder_outputs, embedding,
                          bigram_matrix, affect_matrix)
    res = run_bass_kernel_spmd(nc, in_maps, core_ids=list(range(NCORES)),
                               **(_spmd_kwargs or {}))
    outp = np.empty((B, 1, S), dtype=np.float32)
    for co in range(NCORES):
        outp[co * BC:(co + 1) * BC, 0, :] = res.results[co]["out"]
    if _want_results:
        return outp, res
    return outp
